# revision 7
# baseline (speedup 1.0000x reference)
"""MultiHeadAxialAttention TRN2 kernel (v2).

Problem: x[4,128,128,512] -> 1x1 conv q/k/v projections -> axial attention
(column attention over H, then row attention over W, per head) -> [4,128,128,512].

Sharding: core = (batch b, head-group of 4 heads); 8 cores, zero cross-core
communication. Host pre-transposes x[b] to x^T [512, 16384] so the device
never transposes x; host reassembles the [n, w, h, d]-laid-out per-core
outputs into the reference channel order (channel = d*8 + n).

v2 redesign vs v1 (772us):
  - per-wb softmax normalization (reciprocal + one TENSOR_TENSOR psum->sbuf)
    instead of deferred sums/transposed-AP TT monsters (2.5us each).
  - two heads of a group processed in lockstep: their K=64 score matmuls
    auto-assign to PE row-tiles T0/T8 (64x128 mode) and run concurrently.
  - full-partition V rearrange (128-wide transposes, half the instruction
    count of v1's 64-wide).
  - clip offloaded to GpSimd (was VectorE), balancing engines.
  - xv2 stored [w, d, h] so its psum evacuation is a contiguous copy; the
    row-pass matmul reads it with a strided rhs AP instead.
  - bf16 output DMA (host casts to f32).
"""
import sys
import os
import math

import numpy as np
import ml_dtypes

if "/opt/trn_rl_repo" not in sys.path:
    sys.path.insert(0, "/opt/trn_rl_repo")

B, H, W, C = 4, 128, 128, 512
NH, D = 8, 64
NCORES = 8
NGROUPS = 2          # head groups per core, 2 heads each
PIX = H * W          # 16384, h-major (pix = h*128 + w)
CLIP = 1.0 - 1e-7
SCALE = 1.0 / math.sqrt(D)   # 1/8
EXP_LO = float(np.float32(math.exp(-CLIP * SCALE)))
EXP_HI = float(np.float32(math.exp(CLIP * SCALE)))

_CACHE = {}


def _build_bass():
    import concourse.bacc as bacc
    import concourse.tile as tile
    import concourse.mybir as mybir
    from concourse import masks

    F32 = mybir.dt.float32
    BF16 = mybir.dt.bfloat16
    Act = mybir.ActivationFunctionType
    Alu = mybir.AluOpType

    nc = bacc.Bacc(None, target_bir_lowering=False)

    # DRAM I/O (per-core shapes; SPMD over in_maps)
    xT_d = nc.dram_tensor("xT", [4, 128, PIX], BF16, kind="ExternalInput")
    wq_d = nc.dram_tensor("wq", [4, 128, 256], BF16, kind="ExternalInput")
    wk_d = nc.dram_tensor("wk", [4, 128, 256], BF16, kind="ExternalInput")
    wv_d = nc.dram_tensor("wv", [4, 128, 256], BF16, kind="ExternalInput")
    bq_d = nc.dram_tensor("bq", [128, 2], F32, kind="ExternalInput")
    bk_d = nc.dram_tensor("bk", [128, 2], F32, kind="ExternalInput")
    bv_d = nc.dram_tensor("bv", [128, 2], F32, kind="ExternalInput")
    out_d = nc.dram_tensor("out", [4, W, H, D], BF16, kind="ExternalOutput")

    with tile.TileContext(nc) as tc:
        with (
            tc.tile_pool(name="const", bufs=1) as constp,
            tc.tile_pool(name="persist", bufs=1) as persist,
            tc.tile_pool(name="xt", bufs=3) as xtp,
            tc.tile_pool(name="ebuf", bufs=3) as ebufp,
            tc.tile_pool(name="rsbuf", bufs=2) as rsp,
            tc.tile_pool(name="obuf", bufs=3) as obufp,
            tc.tile_pool(name="ps", bufs=2, space="PSUM") as psp,
        ):
            ident_bf16 = constp.tile([128, 128], BF16, tag="id16")
            masks.make_identity(nc, ident_bf16[:])

            wsb = {}
            bsb = {}
            for nm, wd, bd in (("q", wq_d, bq_d), ("k", wk_d, bk_d),
                               ("v", wv_d, bv_d)):
                wt = constp.tile([128, 4, 256], BF16, tag=f"w{nm}")
                for kc in range(4):
                    nc.sync.dma_start(wt[:, kc, :], wd[kc])
                bt = constp.tile([128, 2], F32, tag=f"b{nm}")
                nc.sync.dma_start(bt[:], bd[:])
                wsb[nm] = wt
                bsb[nm] = bt

            # persistent per-group tensors
            QT = persist.tile([128, PIX], BF16, tag="QT")
            KT = persist.tile([128, PIX], BF16, tag="KT")

            NT = PIX // 512   # 32 pixel tiles of 512

            KREPS = int(os.environ.get("KREPS", "1"))
            for rep in range(KREPS):
              for g in range(NGROUPS):
                  fsl = slice(g * 128, (g + 1) * 128)

                  # VTh is dead after phase A2 and xv_sb is first written in
                  # phase B; same for V_sb (dead after B) and xv2 (written in
                  # C) — alias each pair through a shared bufs=1 pool tag.
                  VTh = persist.tile([128, PIX], BF16, tag="big", name="VTh")
                  V_sb = [persist.tile([128, W, D + 1], BF16, tag=f"Vx{j}",
                                       name=f"V{j}") for j in range(2)]

                  # ---- phase A: projections ----
                  for tt in range(NT):
                      xt = xtp.tile([128, 4, 512], BF16, tag="xt")
                      nc.sync.dma_start(
                          xt[:],
                          xT_d[:, :, tt * 512:(tt + 1) * 512].transpose(
                              [1, 0, 2]))
                      for nm, dst in (("q", QT), ("k", KT), ("v", VTh)):
                          ps = psp.tile([128, 512], F32, tag="A")
                          for kc in range(4):
                              nc.tensor.matmul(
                                  ps[:], wsb[nm][:, kc, fsl], xt[:, kc, :],
                                  start=(kc == 0), stop=(kc == 3))
                          dslice = dst[:, tt * 512:(tt + 1) * 512]
                          if nm == "k":
                              nc.vector.tensor_scalar(
                                  dslice, ps[:], bsb[nm][:, g:g + 1], None,
                                  Alu.add)
                          else:
                              nc.scalar.activation(
                                  dslice, ps[:], Act.Identity,
                                  bias=bsb[nm][:, g:g + 1], scale=1.0)

                  # ---- phase A2: V rearrange V^T[f, pix] -> V_sb[h, w, d] ----
                  for wb in range(16):
                      tps = psp.tile([128, 8, 128], BF16, tag="B")
                      for j in range(8):
                          w = wb * 8 + j
                          nc.tensor.transpose(
                              tps[:, j, :], VTh[:, w::128], ident_bf16[:])
                      for jh in range(2):
                          nc.vector.tensor_copy(
                              V_sb[jh][:, wb * 8:wb * 8 + 8, 0:D],
                              tps[:, :, jh * 64:(jh + 1) * 64])
                  for jh in range(2):
                      nc.vector.memset(V_sb[jh][:, :, D], 1.0)

                  # ---- phase B: column attention, heads paired ----
                  # wb covers 4 columns x 2 heads; score matmuls for jh=0/1
                  # land on PE row-tiles T0/T8 (64x128) and run concurrently.
                  xv_sb = persist.tile([128, W, 2, D], BF16, tag="big",
                                       name="xv_sb")
                  for wb in range(32):
                      sps = psp.tile([128, 8, 128], F32, tag="A")
                      for c in range(4):
                          w = wb * 4 + c
                          for jh in range(2):
                              hsl = slice(jh * 64, (jh + 1) * 64)
                              nc.tensor.matmul(
                                  sps[:, jh * 4 + c, :],
                                  KT[hsl, w::128], QT[hsl, w::128],
                                  start=True, stop=True)
                      ex = ebufp.tile([128, 8, 128], BF16, tag="ex")
                      nc.scalar.activation(ex[:], sps[:], Act.Exp,
                                           scale=SCALE)
                      nc.gpsimd.tensor_scalar(ex[:], ex[:], EXP_LO, EXP_HI,
                                              Alu.max, Alu.min)
                      xvps = psp.tile([128, 8, 128], F32, tag="B")
                      for c in range(4):
                          w = wb * 4 + c
                          for jh in range(2):
                              j = jh * 4 + c
                              nc.tensor.matmul(
                                  xvps[:, j, 0:D + 1],
                                  ex[:, j, :], V_sb[jh][:, w, :],
                                  start=True, stop=True)
                      rs = rsp.tile([128, 8], F32, tag="rs")
                      nc.vector.reciprocal(rs[:], xvps[:, :, D])
                      nc.vector.tensor_tensor(
                          xv_sb[:, wb * 4:wb * 4 + 4, :, :],
                          xvps[:, :, 0:D].rearrange(
                              "p (j c) d -> p c j d", j=2),
                          rs[:].rearrange("p (j c) -> p c j", j=2)
                              .unsqueeze(3).broadcast_to([128, 4, 2, D]),
                          Alu.mult)

                  # ---- phase C: xv [h, w, d] -> xv2 [w, d, h] per head ----
                  xv2 = [persist.tile([128, D + 1, H], BF16, tag=f"Vx{j}",
                                      name=f"xv2_{j}") for j in range(2)]
                  for jh in range(2):
                      for db in range(16):
                          mps = psp.tile([128, 4, 128], BF16, tag="B")
                          for j in range(4):
                              d = db * 4 + j
                              nc.tensor.transpose(
                                  mps[:, j, :], xv_sb[:, :, jh, d],
                                  ident_bf16[:])
                          nc.vector.tensor_copy(
                              xv2[jh][:, db * 4:db * 4 + 4, :], mps[:])
                      nc.vector.memset(xv2[jh][:, D, :], 1.0)

                  # ---- phase D: row attention, heads paired ----
                  for hb in range(32):
                      sps2 = psp.tile([128, 8, 128], F32, tag="A")
                      for c in range(4):
                          h = hb * 4 + c
                          for jh in range(2):
                              hsl = slice(jh * 64, (jh + 1) * 64)
                              nc.tensor.matmul(
                                  sps2[:, jh * 4 + c, :],
                                  KT[hsl, h * 128:(h + 1) * 128],
                                  QT[hsl, h * 128:(h + 1) * 128],
                                  start=True, stop=True)
                      eu = ebufp.tile([128, 8, 128], BF16, tag="ex")
                      nc.scalar.activation(eu[:], sps2[:], Act.Exp,
                                           scale=SCALE)
                      nc.gpsimd.tensor_scalar(eu[:], eu[:], EXP_LO, EXP_HI,
                                              Alu.max, Alu.min)
                      xups = psp.tile([128, 8, 128], F32, tag="B")
                      for c in range(4):
                          h = hb * 4 + c
                          for jh in range(2):
                              j = jh * 4 + c
                              nc.tensor.matmul(
                                  xups[:, j, 0:D + 1],
                                  eu[:, j, :], xv2[jh][:, :, h],
                                  start=True, stop=True)
                      ru = rsp.tile([128, 8], F32, tag="rs")
                      nc.vector.reciprocal(ru[:], xups[:, :, D])
                      ob = obufp.tile([128, 4, 2, D], BF16, tag="ob")
                      nc.vector.tensor_tensor(
                          ob[:],
                          xups[:, :, 0:D].rearrange(
                              "p (j c) d -> p c j d", j=2),
                          ru[:].rearrange("p (j c) -> p c j", j=2)
                              .unsqueeze(3).broadcast_to([128, 4, 2, D]),
                          Alu.mult)
                      for jh in range(2):
                          nc.sync.dma_start(
                              out_d[g * 2 + jh, :, hb * 4:hb * 4 + 4, :],
                              ob[:, :, jh, :])

    nc.compile()
    return nc


def _get_nc():
    if "nc" not in _CACHE:
        _CACHE["nc"] = _build_bass()
    return _CACHE["nc"]


def kernel(x, wq, bq, wk, bk, wv, bv):
    from concourse.bass_utils import run_bass_kernel_spmd

    x = np.asarray(x, dtype=np.float32)
    wq = np.asarray(wq, dtype=np.float32)
    wk = np.asarray(wk, dtype=np.float32)
    wv = np.asarray(wv, dtype=np.float32)
    bq = np.asarray(bq, dtype=np.float32)
    bk = np.asarray(bk, dtype=np.float32)
    bv = np.asarray(bv, dtype=np.float32)

    nc = _get_nc()

    in_maps = []
    for core in range(NCORES):
        b = core // 2
        g2 = core % 2
        heads = list(range(g2 * 4, g2 * 4 + 4))
        cols = np.concatenate(
            [np.arange(n * D, (n + 1) * D) for n in heads])
        xb = x[b].reshape(PIX, C)
        xT = np.ascontiguousarray(xb.T).reshape(4, 128, PIX)
        in_maps.append({
            "xT": xT.astype(ml_dtypes.bfloat16),
            "wq": np.ascontiguousarray(wq[:, cols]).reshape(
                4, 128, 256).astype(ml_dtypes.bfloat16),
            "wk": np.ascontiguousarray(wk[:, cols]).reshape(
                4, 128, 256).astype(ml_dtypes.bfloat16),
            "wv": np.ascontiguousarray(wv[:, cols]).reshape(
                4, 128, 256).astype(ml_dtypes.bfloat16),
            "bq": np.ascontiguousarray(bq[cols].reshape(2, 128).T),
            "bk": np.ascontiguousarray(bk[cols].reshape(2, 128).T),
            "bv": np.ascontiguousarray(bv[cols].reshape(2, 128).T),
        })

    res = run_bass_kernel_spmd(nc, in_maps, list(range(NCORES)),
                               trace=bool(os.environ.get("KTRACE")))
    _CACHE["last_results"] = res

    out = np.empty((B, H, W, C), dtype=np.float32)
    for core in range(NCORES):
        r = np.asarray(res.results[core]["out"], dtype=np.float32)
        b = core // 2
        g2 = core % 2
        for jn, n in enumerate(range(g2 * 4, g2 * 4 + 4)):
            # r[jn] is [w, h, d]; reference channel order is d*NH + n
            out[b, :, :, n::NH] = r[jn].transpose(1, 0, 2)
    return out


# revision 8
# speedup vs baseline: 3.8145x; 3.8145x over previous
"""MultiHeadAxialAttention TRN2 kernel (v2).

Problem: x[4,128,128,512] -> 1x1 conv q/k/v projections -> axial attention
(column attention over H, then row attention over W, per head) -> [4,128,128,512].

Sharding: core = (batch b, head-group of 4 heads); 8 cores, zero cross-core
communication. Host pre-transposes x[b] to x^T [512, 16384] so the device
never transposes x; host reassembles the [n, w, h, d]-laid-out per-core
outputs into the reference channel order (channel = d*8 + n).

v2 redesign vs v1 (772us):
  - per-wb softmax normalization (reciprocal + one TENSOR_TENSOR psum->sbuf)
    instead of deferred sums/transposed-AP TT monsters (2.5us each).
  - two heads of a group processed in lockstep: their K=64 score matmuls
    auto-assign to PE row-tiles T0/T8 (64x128 mode) and run concurrently.
  - full-partition V rearrange (128-wide transposes, half the instruction
    count of v1's 64-wide).
  - clip offloaded to GpSimd (was VectorE), balancing engines.
  - xv2 stored [w, d, h] so its psum evacuation is a contiguous copy; the
    row-pass matmul reads it with a strided rhs AP instead.
  - bf16 output DMA (host casts to f32).
"""
import sys
import os
import math

import numpy as np
import ml_dtypes

if "/opt/trn_rl_repo" not in sys.path:
    sys.path.insert(0, "/opt/trn_rl_repo")

B, H, W, C = 4, 128, 128, 512
NH, D = 8, 64
NCORES = 8
NGROUPS = 2          # head groups per core, 2 heads each
PIX = H * W          # 16384, h-major (pix = h*128 + w)
CLIP = 1.0 - 1e-7
SCALE = 1.0 / math.sqrt(D)   # 1/8
EXP_LO = float(np.float32(math.exp(-CLIP * SCALE)))
EXP_HI = float(np.float32(math.exp(CLIP * SCALE)))

_CACHE = {}


def _build_bass():
    import concourse.bacc as bacc
    import concourse.tile as tile
    import concourse.mybir as mybir
    from concourse import masks

    F32 = mybir.dt.float32
    BF16 = mybir.dt.bfloat16
    Act = mybir.ActivationFunctionType
    Alu = mybir.AluOpType

    nc = bacc.Bacc(None, target_bir_lowering=False)

    # DRAM I/O (per-core shapes; SPMD over in_maps)
    xT_d = nc.dram_tensor("xT", [4, 128, PIX], BF16, kind="ExternalInput")
    wq_d = nc.dram_tensor("wq", [4, 128, 256], BF16, kind="ExternalInput")
    wk_d = nc.dram_tensor("wk", [4, 128, 256], BF16, kind="ExternalInput")
    wv_d = nc.dram_tensor("wv", [4, 128, 256], BF16, kind="ExternalInput")
    bq_d = nc.dram_tensor("bq", [128, 2], F32, kind="ExternalInput")
    bk_d = nc.dram_tensor("bk", [128, 2], F32, kind="ExternalInput")
    bv_d = nc.dram_tensor("bv", [128, 2], F32, kind="ExternalInput")
    out_d = nc.dram_tensor("out", [4, W, H, D], BF16, kind="ExternalOutput")

    with tile.TileContext(nc) as tc:
        with (
            tc.tile_pool(name="const", bufs=1) as constp,
            tc.tile_pool(name="persist", bufs=1) as persist,
            tc.tile_pool(name="xt", bufs=3) as xtp,
            tc.tile_pool(name="ebuf", bufs=3) as ebufp,
            tc.tile_pool(name="rsbuf", bufs=2) as rsp,
            tc.tile_pool(name="obuf", bufs=3) as obufp,
            tc.tile_pool(name="ps", bufs=2, space="PSUM") as psp,
        ):
            ident_bf16 = constp.tile([128, 128], BF16, tag="id16")
            masks.make_identity(nc, ident_bf16[:])

            wsb = {}
            bsb = {}
            for nm, wd, bd in (("q", wq_d, bq_d), ("k", wk_d, bk_d),
                               ("v", wv_d, bv_d)):
                wt = constp.tile([128, 4, 256], BF16, tag=f"w{nm}")
                for kc in range(4):
                    nc.sync.dma_start(wt[:, kc, :], wd[kc])
                bt = constp.tile([128, 2], F32, tag=f"b{nm}")
                nc.sync.dma_start(bt[:], bd[:])
                wsb[nm] = wt
                bsb[nm] = bt

            # persistent per-group tensors
            QT = persist.tile([128, PIX], BF16, tag="QT")
            KT = persist.tile([128, PIX], BF16, tag="KT")

            NT = PIX // 512   # 32 pixel tiles of 512

            KREPS = int(os.environ.get("KREPS", "1"))
            for rep in range(KREPS):
              for g in range(NGROUPS):
                  fsl = slice(g * 128, (g + 1) * 128)

                  # VTh is dead after phase A2 and xv_sb is first written in
                  # phase B; same for V_sb (dead after B) and xv2 (written in
                  # C) — alias each pair through a shared bufs=1 pool tag.
                  VTh = persist.tile([128, PIX], BF16, tag="big", name="VTh")
                  V_sb = [persist.tile([128, W, D + 1], BF16, tag=f"Vx{j}",
                                       name=f"V{j}") for j in range(2)]

                  # ---- phase A: projections ----
                  for tt in range(NT):
                      xt = xtp.tile([128, 4, 512], BF16, tag="xt")
                      nc.sync.dma_start(
                          xt[:],
                          xT_d[:, :, tt * 512:(tt + 1) * 512].transpose(
                              [1, 0, 2]))
                      for nm, dst in (("q", QT), ("k", KT), ("v", VTh)):
                          ps = psp.tile([128, 512], F32, tag="A")
                          for kc in range(4):
                              nc.tensor.matmul(
                                  ps[:], wsb[nm][:, kc, fsl], xt[:, kc, :],
                                  start=(kc == 0), stop=(kc == 3))
                          dslice = dst[:, tt * 512:(tt + 1) * 512]
                          if nm == "k":
                              nc.vector.tensor_scalar(
                                  dslice, ps[:], bsb[nm][:, g:g + 1], None,
                                  Alu.add)
                          else:
                              nc.scalar.activation(
                                  dslice, ps[:], Act.Identity,
                                  bias=bsb[nm][:, g:g + 1], scale=1.0)

                  # ---- phase A2: V rearrange V^T[f, pix] -> V_sb[h, w, d] ----
                  for wb in range(16):
                      tps = psp.tile([128, 8, 128], BF16, tag="B")
                      for j in range(8):
                          w = wb * 8 + j
                          nc.tensor.transpose(
                              tps[:, j, :], VTh[:, w::128], ident_bf16[:])
                      for jh in range(2):
                          nc.vector.tensor_copy(
                              V_sb[jh][:, wb * 8:wb * 8 + 8, 0:D],
                              tps[:, :, jh * 64:(jh + 1) * 64])
                  for jh in range(2):
                      nc.vector.memset(V_sb[jh][:, :, D], 1.0)

                  # ---- phase B: column attention, heads paired ----
                  # wb covers 4 columns x 2 heads; score matmuls for jh=0/1
                  # land on PE row-tiles T0/T8 (64x128) and run concurrently.
                  xv_sb = persist.tile([128, W, 2, D], BF16, tag="big",
                                       name="xv_sb")
                  for wb in range(32):
                      sps = psp.tile([128, 8, 128], F32, tag="A")
                      for c in range(4):
                          w = wb * 4 + c
                          for jh in range(2):
                              hsl = slice(jh * 64, (jh + 1) * 64)
                              nc.tensor.matmul(
                                  sps[:, jh * 4 + c, :],
                                  KT[hsl, w::128], QT[hsl, w::128],
                                  start=True, stop=True)
                      ex = ebufp.tile([128, 8, 128], BF16, tag="ex")
                      nc.scalar.activation(ex[:], sps[:], Act.Exp,
                                           scale=SCALE)
                      nc.vector.tensor_scalar(ex[:], ex[:], EXP_LO, EXP_HI,
                                              Alu.max, Alu.min)
                      xvps = psp.tile([128, 8, 128], F32, tag="B")
                      for c in range(4):
                          w = wb * 4 + c
                          for jh in range(2):
                              j = jh * 4 + c
                              nc.tensor.matmul(
                                  xvps[:, j, 0:D + 1],
                                  ex[:, j, :], V_sb[jh][:, w, :],
                                  start=True, stop=True)
                      rs = rsp.tile([128, 8], F32, tag="rs")
                      nc.vector.reciprocal(rs[:], xvps[:, :, D])
                      nc.vector.tensor_tensor(
                          xv_sb[:, wb * 4:wb * 4 + 4, :, :],
                          xvps[:, :, 0:D].rearrange(
                              "p (j c) d -> p c j d", j=2),
                          rs[:].rearrange("p (j c) -> p c j", j=2)
                              .unsqueeze(3).broadcast_to([128, 4, 2, D]),
                          Alu.mult)

                  # ---- phase C: xv [h, w, d] -> xv2 [w, d, h] per head ----
                  xv2 = [persist.tile([128, D + 1, H], BF16, tag=f"Vx{j}",
                                      name=f"xv2_{j}") for j in range(2)]
                  for jh in range(2):
                      for db in range(16):
                          mps = psp.tile([128, 4, 128], BF16, tag="B")
                          for j in range(4):
                              d = db * 4 + j
                              nc.tensor.transpose(
                                  mps[:, j, :], xv_sb[:, :, jh, d],
                                  ident_bf16[:])
                          nc.vector.tensor_copy(
                              xv2[jh][:, db * 4:db * 4 + 4, :], mps[:])
                      nc.vector.memset(xv2[jh][:, D, :], 1.0)

                  # ---- phase D: row attention, heads paired ----
                  for hb in range(32):
                      sps2 = psp.tile([128, 8, 128], F32, tag="A")
                      for c in range(4):
                          h = hb * 4 + c
                          for jh in range(2):
                              hsl = slice(jh * 64, (jh + 1) * 64)
                              nc.tensor.matmul(
                                  sps2[:, jh * 4 + c, :],
                                  KT[hsl, h * 128:(h + 1) * 128],
                                  QT[hsl, h * 128:(h + 1) * 128],
                                  start=True, stop=True)
                      eu = ebufp.tile([128, 8, 128], BF16, tag="ex")
                      nc.scalar.activation(eu[:], sps2[:], Act.Exp,
                                           scale=SCALE)
                      nc.vector.tensor_scalar(eu[:], eu[:], EXP_LO, EXP_HI,
                                              Alu.max, Alu.min)
                      xups = psp.tile([128, 8, 128], F32, tag="B")
                      for c in range(4):
                          h = hb * 4 + c
                          for jh in range(2):
                              j = jh * 4 + c
                              nc.tensor.matmul(
                                  xups[:, j, 0:D + 1],
                                  eu[:, j, :], xv2[jh][:, :, h],
                                  start=True, stop=True)
                      ru = rsp.tile([128, 8], F32, tag="rs")
                      nc.vector.reciprocal(ru[:], xups[:, :, D])
                      ob = obufp.tile([128, 4, 2, D], BF16, tag="ob")
                      nc.vector.tensor_tensor(
                          ob[:],
                          xups[:, :, 0:D].rearrange(
                              "p (j c) d -> p c j d", j=2),
                          ru[:].rearrange("p (j c) -> p c j", j=2)
                              .unsqueeze(3).broadcast_to([128, 4, 2, D]),
                          Alu.mult)
                      for jh in range(2):
                          nc.sync.dma_start(
                              out_d[g * 2 + jh, :, hb * 4:hb * 4 + 4, :],
                              ob[:, :, jh, :])

    nc.compile()
    return nc


def _get_nc():
    if "nc" not in _CACHE:
        _CACHE["nc"] = _build_bass()
    return _CACHE["nc"]


def kernel(x, wq, bq, wk, bk, wv, bv):
    from concourse.bass_utils import run_bass_kernel_spmd

    x = np.asarray(x, dtype=np.float32)
    wq = np.asarray(wq, dtype=np.float32)
    wk = np.asarray(wk, dtype=np.float32)
    wv = np.asarray(wv, dtype=np.float32)
    bq = np.asarray(bq, dtype=np.float32)
    bk = np.asarray(bk, dtype=np.float32)
    bv = np.asarray(bv, dtype=np.float32)

    nc = _get_nc()

    in_maps = []
    for core in range(NCORES):
        b = core // 2
        g2 = core % 2
        heads = list(range(g2 * 4, g2 * 4 + 4))
        cols = np.concatenate(
            [np.arange(n * D, (n + 1) * D) for n in heads])
        xb = x[b].reshape(PIX, C)
        xT = np.ascontiguousarray(xb.T).reshape(4, 128, PIX)
        in_maps.append({
            "xT": xT.astype(ml_dtypes.bfloat16),
            "wq": np.ascontiguousarray(wq[:, cols]).reshape(
                4, 128, 256).astype(ml_dtypes.bfloat16),
            "wk": np.ascontiguousarray(wk[:, cols]).reshape(
                4, 128, 256).astype(ml_dtypes.bfloat16),
            "wv": np.ascontiguousarray(wv[:, cols]).reshape(
                4, 128, 256).astype(ml_dtypes.bfloat16),
            "bq": np.ascontiguousarray(bq[cols].reshape(2, 128).T),
            "bk": np.ascontiguousarray(bk[cols].reshape(2, 128).T),
            "bv": np.ascontiguousarray(bv[cols].reshape(2, 128).T),
        })

    res = run_bass_kernel_spmd(nc, in_maps, list(range(NCORES)),
                               trace=bool(os.environ.get("KTRACE")))
    _CACHE["last_results"] = res

    out = np.empty((B, H, W, C), dtype=np.float32)
    for core in range(NCORES):
        r = np.asarray(res.results[core]["out"], dtype=np.float32)
        b = core // 2
        g2 = core % 2
        for jn, n in enumerate(range(g2 * 4, g2 * 4 + 4)):
            # r[jn] is [w, h, d]; reference channel order is d*NH + n
            out[b, :, :, n::NH] = r[jn].transpose(1, 0, 2)
    return out
